# revision 19
# baseline (speedup 1.0000x reference)
"""CPAMDec attention-decoder kernel for 8 Trainium2 NeuronCores.

Reference computation (per batch n of N=8):
    q  = x_n^T @ wq^T + bq          (HW=4096, C4=128)
    k  = y_n @ wk^T + bk            (K=32, C4=128)
    v  = y_n @ wv^T + bv            (K=32, C=512)
    attn = softmax(q @ k^T, axis=-1)        (HW, K)
    out = scale * (v^T @ attn^T) + x_n      (C, HW)

Sharding: pure data parallel - core i computes batch i. Params are
replicated; the host packs them into three SBUF-image mega-tensors so
they arrive in 3 large DMAs (one issue each) instead of 17 small ones.

Structure: column-streaming pipeline over 8 chunks of 512 pixels.
x arrives in 5 DMAs (1MB, 1MB, 2MB, 2MB, 2MB), all issued up-front on
the scalar HWDGE ring; output leaves in 8 strided 1MB stores on the
sync ring. Each chunk runs q -> energy -> exp -> sum -> 1/sum -> attn
-> out-matmul -> epilogue -> store, software-pipelined with a 3-step
skew so input DMA, compute on all engines, and output DMA overlap.

Bias/residual folding:
  - bq contributes a per-key bias e_b[j] = sum_o bq[o]*k[j,o], applied
    inside the exp() activation (exact algebra).
  - bv is folded into v via a rank-1 matmul (ones-row x bv-row), using
    sum_j attn[p,j] = 1.
  - the scale s is folded into v; the residual x is added half via DVE
    tensor_tensor (row-tiles 0-1) and half via an identity-matmul PSUM
    accumulation + ACT copy (row-tiles 2-3), balancing DVE/ACT load.

PE warm-up runs off memset tiles (no HBM traffic) so the HAM clock
gate is released before the first real matmul.
"""

import sys

sys.path.insert(0, "/opt/trn_rl_repo")

import numpy as np

import concourse.bacc as bacc
import concourse.mybir as mybir
import concourse.tile as tile
from concourse.bass_utils import run_bass_kernel_spmd

F32 = mybir.dt.float32
F32R = mybir.dt.float32r
AF = mybir.ActivationFunctionType

N, C, H, W, K = 8, 512, 64, 64, 32
HW = H * W            # 4096
C4 = C // 4           # 128
PC = 512              # free-dim chunk (1 PSUM bank of fp32)
NPC = HW // PC        # 8 chunks
KC = C // 128         # 4 contraction chunks
CT = C // 128         # 4 output row-tiles

# megaB column offsets (see _in_maps packing)
MB_WK = 0
MB_YT = 512
MB_BQ = 1024
MB_BK = 1056
MB_ID = 1057
MB_BV = 1185
MB_S = 1697
MB_W = 1698


def _emit(nc, tc):
    sync = nc.sync

    with (
        tc.tile_pool(name="const", bufs=1) as cst,
        tc.tile_pool(name="xbuf", bufs=1) as xp,
        tc.tile_pool(name="work", bufs=3) as wk_pool,
        tc.tile_pool(name="ps", bufs=2, space="PSUM") as ps,
    ):
        # PE warm-up off memset tiles: the HAM clock gate only unthrottles
        # (1.2 -> 2.4 GHz) after ~3.4us of sustained matmul activity. Burn
        # the input-DMA window with dummy matmuls so real ones run warm.
        dmy_w = cst.tile([128, 128], F32, name="dmy_w", tag="dmy_w")
        nc.gpsimd.memset(dmy_w[:], 0.0)
        dmy_x = cst.tile([128, PC], F32, name="dmy_x", tag="dmy_x")
        nc.gpsimd.memset(dmy_x[:], 0.0)
        ones32 = cst.tile([K, 128], F32, name="ones32", tag="ones32")
        nc.gpsimd.memset(ones32[:], 1.0)
        dmy_ps = ps.tile([128, PC], F32, name="dmy_ps", tag="q", bufs=2)
        for _ in range(11):
            nc.tensor.matmul(dmy_ps[:], dmy_w[:].bitcast(F32R),
                             dmy_x[:].bitcast(F32R), start=True, stop=True)

        # x column chunks, issued up-front on the scalar HWDGE ring so it
        # never drains: 2 x 1MB (quick pipeline start) + 3 x 2MB.
        xtiles = []
        for i, (c0, c1) in enumerate(
                [(0, 512), (512, 1024), (1024, 2048), (2048, 3072),
                 (3072, 4096)]):
            t = xp.tile([128, KC, c1 - c0], F32R, name=f"xs{i}",
                        tag=f"xs{i}")
            src = nc.t.x[:, c0:c1].bitcast(F32R).rearrange(
                "(k p) f -> p k f", p=128)
            nc.scalar.dma_start(t[:], src)
            xtiles.append((t, c0))

        def xsl(pc):
            """(tile, free-offset) holding pixel columns of chunk pc."""
            i = pc if pc < 2 else 2 + (pc - 2) // 2
            t, c0 = xtiles[i]
            return t, pc * PC - c0

        # params: three mega-DMAs on the sync ring (wq first - q(0) needs
        # it; wv last - first needed by stage_out(0) at ~step 3).
        mgA = cst.tile([128, 512], F32R, name="mgA", tag="mgA")
        sync.dma_start(mgA[:], nc.t.megaA[:].bitcast(F32R))
        mgB = cst.tile([128, MB_W], F32R, name="mgB", tag="mgB")
        sync.dma_start(mgB[:], nc.t.megaB[:].bitcast(F32R))
        mgC = cst.tile([128, 4 * C], F32R, name="mgC", tag="mgC")
        sync.dma_start(mgC[:], nc.t.megaC[:].bitcast(F32R))

        def wq(k):
            return mgA[:, 128 * k:128 * (k + 1)]

        def wkt(k):
            return mgB[:, MB_WK + 128 * k:MB_WK + 128 * (k + 1)]

        def yt(k):
            return mgB[:, MB_YT + 128 * k:MB_YT + 128 * (k + 1)]

        def wv(k):
            return mgC[:, 512 * k:512 * (k + 1)]

        pro = {}

        def emit_kt_eb():
            # kT (with bk) and the bq-driven per-key energy bias.
            kt_ps = ps.tile([C4, 4 * K], F32, name="kt_ps", tag="e", bufs=1)
            for k in range(KC):
                nc.tensor.matmul(kt_ps[:], wkt(k), yt(k),
                                 start=(k == 0), stop=(k == KC - 1))
            ktb4 = cst.tile([C4, 4 * K], F32R, name="ktb4", tag="ktb4")
            nc.vector.tensor_scalar_add(ktb4[:], kt_ps[:],
                                        mgB[:, MB_BK:MB_BK + 1].bitcast(F32))
            eb_ps = ps.tile([4 * K, K], F32, name="eb_ps", tag="o", bufs=2)
            nc.tensor.matmul(eb_ps[:], ktb4[:],
                             mgB[:, MB_BQ:MB_BQ + K],
                             start=True, stop=True)
            e_b4 = cst.tile([4 * K, 1], F32, name="e_b4", tag="e_b4")
            nc.vector.tensor_copy(e_b4[:], eb_ps[:, 0:1])
            pro.update(ktb4=ktb4, e_b4=e_b4)

        def emit_v():
            # v (with bv via rank-1 matmul, scaled by s), partition-stacked.
            v_ps = ps.tile([K, C], F32, name="v_ps", tag="s", bufs=1)
            for k in range(KC):
                nc.tensor.matmul(v_ps[:], yt(k)[:, 0:K], wv(k),
                                 start=(k == 0), stop=False)
            nc.tensor.matmul(v_ps[:], ones32[0:1, 0:K].bitcast(F32R),
                             mgB[0:1, MB_BV:MB_BV + C],
                             start=False, stop=True)
            v_sb = cst.tile([K, C], F32R, name="v_sb", tag="v_sb")
            nc.scalar.activation(out=v_sb[:], in_=v_ps[:], func=AF.Copy,
                                 bias=0.0,
                                 scale=mgB[0:K, MB_S:MB_S + 1].bitcast(F32))
            # vstack[32*ct + j, m] = v_sb[j, 128*ct + m]
            vstack = cst.tile([128, 128], F32R, name="vstack", tag="vstack")
            for ct in range(CT):
                nc.gpsimd.dma_start(
                    vstack[32 * ct:32 * (ct + 1), :],
                    v_sb[:, 128 * ct:128 * (ct + 1)])
            pro.update(vstack=vstack)

        # ------------- software-pipelined main loop over column chunks ----
        # Stages are skewed so every PE instruction's inputs were produced
        # in an earlier iteration (the engine queues are in-order; without
        # the skew the PE stalls mid-chain waiting on ACT/DVE/GpSimd).
        qtcs = [None] * NPC
        expts = [None] * NPC
        attns = [None] * NPC

        def stage_q(pc):
            xt, off = xsl(pc)
            q_ps = ps.tile([C4, PC], F32, name=f"q_ps{pc}", tag="q", bufs=2)
            for k in range(KC):
                nc.tensor.matmul(q_ps[:], wq(k), xt[:, k, off:off + PC],
                                 start=(k == 0), stop=(k == KC - 1))
            qtc = wk_pool.tile([C4, PC], F32R, name="qtc", tag="qtc", bufs=4)
            if pc < 3:
                nc.vector.tensor_copy(qtc[:], q_ps[:])
            else:
                nc.scalar.activation(out=qtc[:], in_=q_ps[:], func=AF.Copy,
                                     scale=1.0)
            qtcs[pc] = qtc

        def stage_energy(pc):
            e_ps = ps.tile([128, PC], F32, name=f"e_ps{pc}", tag="e", bufs=1)
            nc.tensor.matmul(e_ps[:], pro['ktb4'][:], qtcs[pc][:],
                             start=True, stop=True)
            expt = wk_pool.tile([128, PC], F32R, name="expt", tag="expt",
                                bufs=4)
            nc.scalar.activation(out=expt[:], in_=e_ps[:], func=AF.Exp,
                                 bias=pro['e_b4'][:], scale=1.0)
            expts[pc] = expt

        def stage_softmax(pc):
            s_ps = ps.tile([128, PC], F32, name=f"s_ps{pc}", tag="s", bufs=1)
            nc.tensor.matmul(s_ps[:], ones32[:].bitcast(F32R),
                             expts[pc][0:K, :], start=True, stop=True)
            rec = wk_pool.tile([128, PC], F32, name="rec", tag="rec", bufs=4)
            nc.vector.reciprocal_approx_fast(
                out=rec[:], in_=s_ps[:].bitcast(F32))
            attn = wk_pool.tile([128, PC], F32R, name="attn", tag="attn",
                                bufs=4)
            nc.vector.tensor_mul(attn[:], expts[pc][:].bitcast(F32), rec[:])
            attns[pc] = attn

        def stage_out(pc):
            sl = slice(pc * PC, (pc + 1) * PC)
            xt, off = xsl(pc)
            attn = attns[pc]
            osb = wk_pool.tile([128, CT, PC], F32, name="osb", tag="osb",
                               bufs=3)
            dst = nc.t.out[:, sl].rearrange("(k p) f -> p k f", p=128)
            # two half-chunks: 2 out-matmuls -> DVE TT residual -> store,
            # so each 512KB store drains as soon as its half is ready
            for h in range(2):
                ot = ps.tile([128, 2, PC], F32, name=f"ot{pc}_{h}", tag="o",
                             bufs=2)
                for j, ct in enumerate((2 * h, 2 * h + 1)):
                    nc.tensor.matmul(ot[:, j, :],
                                     pro['vstack'][32 * ct:32 * (ct + 1), :],
                                     attn[32 * ct:32 * (ct + 1), :],
                                     start=True, stop=True,
                                     tile_position=(32 * ct, 0))
                nc.vector.tensor_add(
                    osb[:, 2 * h:2 * h + 2, :], ot[:],
                    xt[:, 2 * h:2 * h + 2, off:off + PC].bitcast(F32))
                sync.dma_start(dst[:, 2 * h:2 * h + 2, :],
                               osb[:, 2 * h:2 * h + 2, :])

        stage_q(0)
        emit_kt_eb()
        for step in range(1, NPC + 3):
            if 0 <= step - 1 < NPC:
                stage_energy(step - 1)
            if 0 <= step - 2 < NPC:
                stage_softmax(step - 2)
            if 0 <= step - 3 < NPC:
                stage_out(step - 3)
            if step < NPC:
                stage_q(step)
            if step == 2:
                emit_v()


class _T:
    """Attribute access to declared dram params."""
    def __init__(self):
        self.__dict__ = {}


_NC_CACHE = []


def _build():
    if _NC_CACHE:
        return _NC_CACHE[0]
    nc = bacc.Bacc(target_bir_lowering=False)
    nc.t = _T()
    t = nc.t
    t.x = nc.declare_dram_parameter("x", [C, HW], F32, isOutput=False)
    t.megaA = nc.declare_dram_parameter("megaA", [128, 512], F32,
                                        isOutput=False)
    t.megaB = nc.declare_dram_parameter("megaB", [128, MB_W], F32,
                                        isOutput=False)
    t.megaC = nc.declare_dram_parameter("megaC", [128, 4 * C], F32,
                                        isOutput=False)
    t.out = nc.declare_dram_parameter("out", [C, HW], F32, isOutput=True)
    with tile.TileContext(nc) as tc:
        _emit(nc, tc)
    nc.finalize()
    _NC_CACHE.append(nc)
    return nc


def _sbuf_img(mT):
    """[C, F] (contraction-major) -> SBUF image [128, KC*F]."""
    Cdim, F = mT.shape
    return np.ascontiguousarray(
        mT.reshape(KC, 128, F).transpose(1, 0, 2).reshape(128, KC * F),
        dtype=np.float32)


def _in_maps(x, y, wq, bq, wk, bk, wv, bv, scale):
    x = np.ascontiguousarray(x, dtype=np.float32).reshape(N, C, HW)
    megaA = _sbuf_img(np.float32(wq).T)                    # [128, 512]
    wk_img = _sbuf_img(np.float32(wk).T)                   # [128, 512]
    wv_img = _sbuf_img(np.float32(wv).T)                   # [128, 2048]
    yt_img = np.stack([
        _sbuf_img(np.ascontiguousarray(
            np.tile(np.float32(y[i]).T, (1, 4))))          # [128, 512]
        for i in range(N)])
    bq_img = np.broadcast_to(np.float32(bq).reshape(C4, 1), (C4, K))
    bk_img = np.float32(bk).reshape(C4, 1)
    ident = np.eye(128, dtype=np.float32)
    bv_blk = np.zeros((128, C), dtype=np.float32)
    bv_blk[0] = np.float32(bv)
    s_blk = np.broadcast_to(np.float32(scale).reshape(1, 1), (128, 1))
    megaB = [np.concatenate(
        [wk_img, yt_img[i], bq_img, bk_img, ident, bv_blk, s_blk], axis=1)
        for i in range(N)]
    assert megaB[0].shape[1] == MB_W, megaB[0].shape
    return [
        {
            "x": x[i],
            "megaA": megaA,
            "megaB": np.ascontiguousarray(megaB[i], dtype=np.float32),
            "megaC": wv_img,
        }
        for i in range(N)
    ]


def _run(inputs, **kwargs):
    nc = _build()
    return run_bass_kernel_spmd(nc, _in_maps(**inputs),
                                core_ids=list(range(N)), **kwargs)


def kernel(**inputs) -> np.ndarray:
    res = _run(inputs)
    out = np.stack([res.results[i]["out"] for i in range(N)])
    return out.reshape(N, C, H, W).astype(np.float32)


# revision 20
# speedup vs baseline: 1.1867x; 1.1867x over previous
"""CPAMDec attention-decoder kernel for 8 Trainium2 NeuronCores.

Reference computation (per batch n of N=8):
    q  = x_n^T @ wq^T + bq          (HW=4096, C4=128)
    k  = y_n @ wk^T + bk            (K=32, C4=128)
    v  = y_n @ wv^T + bv            (K=32, C=512)
    attn = softmax(q @ k^T, axis=-1)        (HW, K)
    out = scale * (v^T @ attn^T) + x_n      (C, HW)

Sharding: pure data parallel - core i computes batch i. Params are
replicated; the host packs them into three SBUF-image mega-tensors so
they arrive in 3 large DMAs (one issue each) instead of 17 small ones.

Structure: column-streaming pipeline over 8 chunks of 512 pixels.
x arrives in 5 DMAs (1MB, 1MB, 2MB, 2MB, 2MB), all issued up-front on
the scalar HWDGE ring; output leaves in 8 strided 1MB stores on the
sync ring. Each chunk runs q -> energy -> exp -> sum -> 1/sum -> attn
-> out-matmul -> epilogue -> store, software-pipelined with a 3-step
skew so input DMA, compute on all engines, and output DMA overlap.

Bias/residual folding:
  - bq contributes a per-key bias e_b[j] = sum_o bq[o]*k[j,o], applied
    inside the exp() activation (exact algebra).
  - bv is folded into v via a rank-1 matmul (ones-row x bv-row), using
    sum_j attn[p,j] = 1.
  - the scale s is folded into v; the residual x is added half via DVE
    tensor_tensor (row-tiles 0-1) and half via an identity-matmul PSUM
    accumulation + ACT copy (row-tiles 2-3), balancing DVE/ACT load.

PE warm-up runs off memset tiles (no HBM traffic) so the HAM clock
gate is released before the first real matmul.
"""

import sys

sys.path.insert(0, "/opt/trn_rl_repo")

import numpy as np

import concourse.bacc as bacc
import concourse.mybir as mybir
import concourse.tile as tile
from concourse.bass_utils import run_bass_kernel_spmd

F32 = mybir.dt.float32
F32R = mybir.dt.float32r
AF = mybir.ActivationFunctionType

N, C, H, W, K = 8, 512, 64, 64, 32
HW = H * W            # 4096
C4 = C // 4           # 128
PC = 512              # free-dim chunk (1 PSUM bank of fp32)
NPC = HW // PC        # 8 chunks
KC = C // 128         # 4 contraction chunks
CT = C // 128         # 4 output row-tiles

# megaB column offsets (see _in_maps packing)
MB_WK = 0
MB_YT = 512
MB_BQ = 1024
MB_BK = 1056
MB_ID = 1057
MB_BV = 1185
MB_S = 1697
MB_W = 1698


def _emit(nc, tc):
    sync = nc.sync

    with (
        tc.tile_pool(name="const", bufs=1) as cst,
        tc.tile_pool(name="xbuf", bufs=1) as xp,
        tc.tile_pool(name="work", bufs=3) as wk_pool,
        tc.tile_pool(name="ps", bufs=2, space="PSUM") as ps,
    ):
        # PE warm-up off memset tiles: the HAM clock gate only unthrottles
        # (1.2 -> 2.4 GHz) after ~3.4us of sustained matmul activity. Burn
        # the input-DMA window with dummy matmuls so real ones run warm.
        dmy_w = cst.tile([128, 128], F32, name="dmy_w", tag="dmy_w")
        nc.gpsimd.memset(dmy_w[:], 0.0)
        dmy_x = cst.tile([128, PC], F32, name="dmy_x", tag="dmy_x")
        nc.gpsimd.memset(dmy_x[:], 0.0)
        ones32 = cst.tile([K, 128], F32, name="ones32", tag="ones32")
        nc.gpsimd.memset(ones32[:], 1.0)
        dmy_ps = ps.tile([128, PC], F32, name="dmy_ps", tag="q", bufs=2)
        for _ in range(14):
            nc.tensor.matmul(dmy_ps[:], dmy_w[:].bitcast(F32R),
                             dmy_x[:].bitcast(F32R), start=True, stop=True)

        # x column chunks, issued up-front on the scalar HWDGE ring so it
        # never drains: 2 x 1MB (quick pipeline start) + 3 x 2MB.
        xtiles = []
        for i, (c0, c1) in enumerate(
                [(0, 512), (512, 1024), (1024, 2048), (2048, 3072),
                 (3072, 4096)]):
            t = xp.tile([128, KC, c1 - c0], F32R, name=f"xs{i}",
                        tag=f"xs{i}")
            src = nc.t.x[:, c0:c1].bitcast(F32R).rearrange(
                "(k p) f -> p k f", p=128)
            nc.scalar.dma_start(t[:], src)
            xtiles.append((t, c0))

        def xsl(pc):
            """(tile, free-offset) holding pixel columns of chunk pc."""
            i = pc if pc < 2 else 2 + (pc - 2) // 2
            t, c0 = xtiles[i]
            return t, pc * PC - c0

        # params: three mega-DMAs on the sync ring (wq first - q(0) needs
        # it; wv last - first needed by stage_out(0) at ~step 3).
        mgA = cst.tile([128, 512], F32R, name="mgA", tag="mgA")
        sync.dma_start(mgA[:], nc.t.megaA[:].bitcast(F32R))
        mgB = cst.tile([128, MB_W], F32R, name="mgB", tag="mgB")
        sync.dma_start(mgB[:], nc.t.megaB[:].bitcast(F32R))
        mgC = cst.tile([128, 4 * C], F32R, name="mgC", tag="mgC")
        sync.dma_start(mgC[:], nc.t.megaC[:].bitcast(F32R))

        def wq(k):
            return mgA[:, 128 * k:128 * (k + 1)]

        def wkt(k):
            return mgB[:, MB_WK + 128 * k:MB_WK + 128 * (k + 1)]

        def yt(k):
            return mgB[:, MB_YT + 128 * k:MB_YT + 128 * (k + 1)]

        def wv(k):
            return mgC[:, 512 * k:512 * (k + 1)]

        pro = {}

        def emit_kt_eb():
            # kT (with bk) and the bq-driven per-key energy bias.
            kt_ps = ps.tile([C4, 4 * K], F32, name="kt_ps", tag="e", bufs=1)
            for k in range(KC):
                nc.tensor.matmul(kt_ps[:], wkt(k), yt(k),
                                 start=(k == 0), stop=(k == KC - 1))
            ktb4 = cst.tile([C4, 4 * K], F32R, name="ktb4", tag="ktb4")
            nc.scalar.activation(out=ktb4[:], in_=kt_ps[:], func=AF.Identity,
                                 bias=mgB[:, MB_BK:MB_BK + 1].bitcast(F32), scale=1.0)
            eb_ps = ps.tile([4 * K, K], F32, name="eb_ps", tag="o", bufs=2)
            nc.tensor.matmul(eb_ps[:], ktb4[:],
                             mgB[:, MB_BQ:MB_BQ + K],
                             start=True, stop=True)
            e_b4 = cst.tile([4 * K, 1], F32, name="e_b4", tag="e_b4")
            nc.scalar.activation(out=e_b4[:], in_=eb_ps[:, 0:1],
                                 func=AF.Copy, scale=1.0)
            pro.update(ktb4=ktb4, e_b4=e_b4)

        def emit_v():
            # v (with bv via rank-1 matmul, scaled by s), partition-stacked.
            v_ps = ps.tile([K, C], F32, name="v_ps", tag="s", bufs=1)
            for k in range(KC):
                nc.tensor.matmul(v_ps[:], yt(k)[:, 0:K], wv(k),
                                 start=(k == 0), stop=False)
            nc.tensor.matmul(v_ps[:], ones32[0:1, 0:K].bitcast(F32R),
                             mgB[0:1, MB_BV:MB_BV + C],
                             start=False, stop=True)
            v_sb = cst.tile([K, C], F32R, name="v_sb", tag="v_sb")
            nc.scalar.activation(out=v_sb[:], in_=v_ps[:], func=AF.Copy,
                                 bias=0.0, scale=mgB[0:K, MB_S:MB_S + 1].bitcast(F32))
            # vstack[32*ct + j, m] = v_sb[j, 128*ct + m]
            vstack = cst.tile([128, 128], F32R, name="vstack", tag="vstack")
            for ct in range(CT):
                nc.gpsimd.dma_start(
                    vstack[32 * ct:32 * (ct + 1), :],
                    v_sb[:, 128 * ct:128 * (ct + 1)])
            pro.update(vstack=vstack)

        # ------------- software-pipelined main loop over column chunks ----
        # Stages are skewed so every PE instruction's inputs were produced
        # in an earlier iteration (the engine queues are in-order; without
        # the skew the PE stalls mid-chain waiting on ACT/DVE/GpSimd).
        qtcs = [None] * NPC
        expts = [None] * NPC
        attns = [None] * NPC

        def stage_q(pc):
            xt, off = xsl(pc)
            q_ps = ps.tile([C4, PC], F32, name=f"q_ps{pc}", tag="q", bufs=2)
            for k in range(KC):
                nc.tensor.matmul(q_ps[:], wq(k), xt[:, k, off:off + PC],
                                 start=(k == 0), stop=(k == KC - 1))
            qtc = wk_pool.tile([C4, PC], F32R, name="qtc", tag="qtc", bufs=4)
            nc.scalar.activation(out=qtc[:], in_=q_ps[:], func=AF.Copy,
                                 scale=1.0)
            qtcs[pc] = qtc

        def stage_energy(pc):
            e_ps = ps.tile([128, PC], F32, name=f"e_ps{pc}", tag="e", bufs=1)
            nc.tensor.matmul(e_ps[:], pro['ktb4'][:], qtcs[pc][:],
                             start=True, stop=True)
            expt = wk_pool.tile([128, PC], F32R, name="expt", tag="expt",
                                bufs=4)
            nc.scalar.activation(out=expt[:], in_=e_ps[:], func=AF.Exp,
                                 bias=pro['e_b4'][:], scale=1.0)
            expts[pc] = expt

        def stage_softmax(pc):
            s_ps = ps.tile([128, PC], F32, name=f"s_ps{pc}", tag="s", bufs=1)
            nc.tensor.matmul(s_ps[:], ones32[:].bitcast(F32R),
                             expts[pc][0:K, :], start=True, stop=True)
            rec = wk_pool.tile([128, PC], F32, name="rec", tag="rec", bufs=4)
            nc.vector.reciprocal_approx_fast(
                out=rec[:], in_=s_ps[:].bitcast(F32))
            attn = wk_pool.tile([128, PC], F32R, name="attn", tag="attn",
                                bufs=4)
            nc.vector.tensor_mul(attn[:], expts[pc][:].bitcast(F32), rec[:])
            attns[pc] = attn

        def stage_out(pc):
            sl = slice(pc * PC, (pc + 1) * PC)
            xt, off = xsl(pc)
            attn = attns[pc]
            osb = wk_pool.tile([128, CT, PC], F32, name="osb", tag="osb",
                               bufs=3)
            dst = nc.t.out[:, sl].rearrange("(k p) f -> p k f", p=128)
            # two half-chunks: 2 out-matmuls -> DVE TT residual -> store,
            # so each 512KB store drains as soon as its half is ready
            for h in range(2):
                ot = ps.tile([128, 2, PC], F32, name=f"ot{pc}_{h}", tag="o",
                             bufs=2)
                for j, ct in enumerate((2 * h, 2 * h + 1)):
                    nc.tensor.matmul(ot[:, j, :],
                                     pro['vstack'][32 * ct:32 * (ct + 1), :],
                                     attn[32 * ct:32 * (ct + 1), :],
                                     start=True, stop=True,
                                     tile_position=(32 * ct, 0))
                nc.vector.tensor_add(
                    osb[:, 2 * h:2 * h + 2, :], ot[:],
                    xt[:, 2 * h:2 * h + 2, off:off + PC].bitcast(F32))
                sync.dma_start(dst[:, 2 * h:2 * h + 2, :],
                               osb[:, 2 * h:2 * h + 2, :])

        stage_q(0)
        emit_kt_eb()
        for step in range(1, NPC + 3):
            if 0 <= step - 1 < NPC:
                stage_energy(step - 1)
            if 0 <= step - 2 < NPC:
                stage_softmax(step - 2)
            if 0 <= step - 3 < NPC:
                stage_out(step - 3)
            if step < NPC:
                stage_q(step)
            if step == 1:
                emit_v()


class _T:
    """Attribute access to declared dram params."""
    def __init__(self):
        self.__dict__ = {}


_NC_CACHE = []


def _build():
    if _NC_CACHE:
        return _NC_CACHE[0]
    nc = bacc.Bacc(target_bir_lowering=False)
    nc.t = _T()
    t = nc.t
    t.x = nc.declare_dram_parameter("x", [C, HW], F32, isOutput=False)
    t.megaA = nc.declare_dram_parameter("megaA", [128, 512], F32,
                                        isOutput=False)
    t.megaB = nc.declare_dram_parameter("megaB", [128, MB_W], F32,
                                        isOutput=False)
    t.megaC = nc.declare_dram_parameter("megaC", [128, 4 * C], F32,
                                        isOutput=False)
    t.out = nc.declare_dram_parameter("out", [C, HW], F32, isOutput=True)
    with tile.TileContext(nc) as tc:
        _emit(nc, tc)
    nc.finalize()
    _NC_CACHE.append(nc)
    return nc


def _sbuf_img(mT):
    """[C, F] (contraction-major) -> SBUF image [128, KC*F]."""
    Cdim, F = mT.shape
    return np.ascontiguousarray(
        mT.reshape(KC, 128, F).transpose(1, 0, 2).reshape(128, KC * F),
        dtype=np.float32)


def _in_maps(x, y, wq, bq, wk, bk, wv, bv, scale):
    x = np.ascontiguousarray(x, dtype=np.float32).reshape(N, C, HW)
    megaA = _sbuf_img(np.float32(wq).T)                    # [128, 512]
    wk_img = _sbuf_img(np.float32(wk).T)                   # [128, 512]
    wv_img = _sbuf_img(np.float32(wv).T)                   # [128, 2048]
    yt_img = np.stack([
        _sbuf_img(np.ascontiguousarray(
            np.tile(np.float32(y[i]).T, (1, 4))))          # [128, 512]
        for i in range(N)])
    bq_img = np.broadcast_to(np.float32(bq).reshape(C4, 1), (C4, K))
    bk_img = np.float32(bk).reshape(C4, 1)
    ident = np.eye(128, dtype=np.float32)
    bv_blk = np.zeros((128, C), dtype=np.float32)
    bv_blk[0] = np.float32(bv)
    s_blk = np.broadcast_to(np.float32(scale).reshape(1, 1), (128, 1))
    megaB = [np.concatenate(
        [wk_img, yt_img[i], bq_img, bk_img, ident, bv_blk, s_blk], axis=1)
        for i in range(N)]
    assert megaB[0].shape[1] == MB_W, megaB[0].shape
    return [
        {
            "x": x[i],
            "megaA": megaA,
            "megaB": np.ascontiguousarray(megaB[i], dtype=np.float32),
            "megaC": wv_img,
        }
        for i in range(N)
    ]


def _run(inputs, **kwargs):
    nc = _build()
    return run_bass_kernel_spmd(nc, _in_maps(**inputs),
                                core_ids=list(range(N)), **kwargs)


def kernel(**inputs) -> np.ndarray:
    res = _run(inputs)
    out = np.stack([res.results[i]["out"] for i in range(N)])
    return out.reshape(N, C, H, W).astype(np.float32)
